# revision 14
# baseline (speedup 1.0000x reference)
"""Trainium2 Bass kernel: brute-force closest-point-on-triangle kNN.

Problem: triangles [2,4096,3,3] f32, points [2,16384,3] f32 ->
  (squared distances [2,16384] f32, closest_points [2,16384,3] f32,
   closest_faces [2,16384] int32)

Strategy (data-parallel over points, 8 cores, 4 per batch):
  - Host precomputes, per triangle, polynomial coefficients so that one
    TensorE matmul (K=10 quadratic point-features) produces the 12 per-pair
    quantities of Ericson's closest-point-on-triangle region test:
       d1,d2,d3,d6, d4-d3, d5-d6, va,vb,vc, ap_x,ap_y,ap_z
  - Device (per 128-triangle tile x 512-point chunk): region masks via
    fused compare/and ops, barycentric (lambda,mu) selection via
    copy_predicated chains, squared distance from the residual
    r = ap - lambda*ab - mu*ac, and a running min/argmin per triangle-lane.
  - Host reduces the 128 triangle lanes per point, gathers the winning
    triangle and recomputes the exact closest point/distance with the
    reference formula (fp32), guaranteeing output fidelity.
"""

import os
import numpy as np

F32 = np.float32

B, F, Q = 2, 4096, 16384
NCORES = 8
CORES_PER_BATCH = 4
QPC = Q // CORES_PER_BATCH          # 4096 points per core
NTT = F // 128                      # 32 triangle tiles
FD = 512                            # point chunk (free dim)
NCH = QPC // FD                     # 8 chunks per core
NQ = 12                             # matmul quantities per pair

_CACHE = {}


# ---------------------------------------------------------------- host math
def _tri_precompute(tris_b):
    """Per-triangle coefficients and constants for one batch.

    Returns coef [NTT, 10, NQ*128] f32  (feature k -> partition, quantity q,
    tri-in-tile m -> free q*128+m) and tconst [128, NTT*10] f32
    (partition = tri-in-tile, free = t*10 + const_id) with const order:
    inv_ab2, inv_ac2, inv_cb2, inv_den, ab_xyz, ac_xyz.
    """
    a = tris_b[:, 0].astype(F32)
    b = tris_b[:, 1].astype(F32)
    c = tris_b[:, 2].astype(F32)
    ab = b - a
    ac = c - a
    cb = c - b

    def guarded_inv(x):
        x = x.astype(F32)
        return (1.0 / np.where(np.abs(x) > 1e-12, x, F32(1.0))).astype(F32)

    ab2 = np.sum(ab * ab, -1)
    ac2 = np.sum(ac * ac, -1)
    cb2 = np.sum(cb * cb, -1)
    cr = np.cross(ab.astype(np.float64), ac.astype(np.float64))
    den = np.sum(cr * cr, -1).astype(F32)
    inv_ab2, inv_ac2, inv_cb2, inv_den = (
        guarded_inv(ab2), guarded_inv(ac2), guarded_inv(cb2), guarded_inv(den))

    # coefficient rows over features [1,x,y,z,x2,y2,z2,xy,xz,yz]
    ab64 = ab.astype(np.float64); ac64 = ac.astype(np.float64)
    a64 = a.astype(np.float64); b64 = b.astype(np.float64)
    c64 = c.astype(np.float64); cb64 = cb.astype(np.float64)
    C = np.zeros((F, NQ, 10), dtype=np.float64)

    def aff(q, vec, const):
        C[:, q, 1:4] = vec
        C[:, q, 0] = const

    e1 = -np.sum(ab64 * a64, -1)
    e2 = -np.sum(ac64 * a64, -1)
    e3 = -np.sum(ab64 * b64, -1)
    e6 = -np.sum(ac64 * c64, -1)
    e4 = -np.sum(ac64 * b64, -1)
    e5 = -np.sum(ab64 * c64, -1)
    aff(0, ab64, e1)                      # d1
    aff(1, ac64, e2)                      # d2
    aff(2, ab64, e3)                      # d3
    aff(3, ac64, e6)                      # d6
    aff(4, cb64, -np.sum(cb64 * b64, -1))  # d4-d3
    aff(5, -cb64, np.sum(cb64 * c64, -1))  # d5-d6

    def quad(u, e, v, f):
        out = np.zeros((F, 10), dtype=np.float64)
        out[:, 0] = e * f
        out[:, 1:4] = u * f[:, None] + v * e[:, None]
        out[:, 4:7] = u * v
        out[:, 7] = u[:, 0] * v[:, 1] + u[:, 1] * v[:, 0]
        out[:, 8] = u[:, 0] * v[:, 2] + u[:, 2] * v[:, 0]
        out[:, 9] = u[:, 1] * v[:, 2] + u[:, 2] * v[:, 1]
        return out

    C[:, 6] = quad(ab64, e3, ac64, e6) - quad(ab64, e5, ac64, e4)   # va
    C[:, 7] = quad(ab64, e5, ac64, e2) - quad(ab64, e1, ac64, e6)   # vb
    C[:, 8] = quad(ab64, e1, ac64, e4) - quad(ab64, e3, ac64, e2)   # vc
    for k in range(3):                    # ap_k = p_k - a_k
        C[:, 9 + k, 0] = -a[:, k]
        C[:, 9 + k, 1 + k] = 1.0

    Cf = C.astype(F32)                                   # [F, NQ, 10]
    coef = np.zeros((NTT, 10, NQ * 128), dtype=F32)
    for t in range(NTT):
        blk = Cf[t * 128:(t + 1) * 128]                  # [128, NQ, 10]
        coef[t] = blk.transpose(2, 1, 0).reshape(10, NQ * 128)

    tconst = np.zeros((128, NTT * 10), dtype=F32)
    cs = np.stack([inv_ab2, inv_ac2, inv_cb2, inv_den,
                   ab[:, 0], ab[:, 1], ab[:, 2],
                   ac[:, 0], ac[:, 1], ac[:, 2]], axis=1)  # [F, 10]
    for t in range(NTT):
        tconst[:, t * 10:(t + 1) * 10] = cs[t * 128:(t + 1) * 128]
    return coef, tconst


def _point_feats(pts):
    p = pts.astype(F32)
    x, y, z = p[:, 0], p[:, 1], p[:, 2]
    return np.stack([np.ones_like(x), x, y, z,
                     x * x, y * y, z * z,
                     x * y, x * z, y * z], axis=0).astype(F32)  # [10, n]


def _safe_div(num, den):
    den_s = np.where(np.abs(den) > 1e-12, den, F32(1.0)).astype(F32)
    return (num / den_s).astype(F32)


def _closest_point_host(p, a, b, c):
    """Exact reference formula (fp32) for n points against n triangles."""
    ab = b - a
    ac = c - a
    ap = p - a
    d1 = np.sum(ab * ap, -1)
    d2 = np.sum(ac * ap, -1)
    bp = p - b
    d3 = np.sum(ab * bp, -1)
    d4 = np.sum(ac * bp, -1)
    cpv = p - c
    d5 = np.sum(ab * cpv, -1)
    d6 = np.sum(ac * cpv, -1)
    vc = d1 * d4 - d3 * d2
    vb = d5 * d2 - d1 * d6
    va = d3 * d6 - d5 * d4
    v_ab = np.clip(_safe_div(d1, d1 - d3), 0.0, 1.0).astype(F32)
    w_ac = np.clip(_safe_div(d2, d2 - d6), 0.0, 1.0).astype(F32)
    w_bc = np.clip(_safe_div(d4 - d3, (d4 - d3) + (d5 - d6)), 0.0, 1.0).astype(F32)
    denom = va + vb + vc
    v = _safe_div(vb, denom)
    w = _safe_div(vc, denom)
    res = a + ab * v[..., None] + ac * w[..., None]
    m_bc = (va <= 0) & ((d4 - d3) >= 0) & ((d5 - d6) >= 0)
    m_ac = (vb <= 0) & (d2 >= 0) & (d6 <= 0)
    m_ab = (vc <= 0) & (d1 >= 0) & (d3 <= 0)
    m_c = (d6 >= 0) & (d5 <= d6)
    m_b = (d3 >= 0) & (d4 <= d3)
    m_a = (d1 <= 0) & (d2 <= 0)
    res = np.where(m_bc[..., None], b + w_bc[..., None] * (c - b), res)
    res = np.where(m_ac[..., None], a + w_ac[..., None] * ac, res)
    res = np.where(m_ab[..., None], a + v_ab[..., None] * ab, res)
    res = np.where(m_c[..., None], c, res)
    res = np.where(m_b[..., None], b, res)
    res = np.where(m_a[..., None], a, res)
    return res.astype(F32)


# ---------------------------------------------------------------- bass build
def _build_bass():
    import concourse.bass as bass
    import concourse.bacc as bacc
    import concourse.mybir as mybir
    from concourse.tile import TileContext

    dt = mybir.dt
    Alu = mybir.AluOpType
    Act = mybir.ActivationFunctionType

    nc = bacc.Bacc()
    coef_d = nc.dram_tensor("coef", [NTT, 10, NQ * 128], dt.float32,
                            kind="ExternalInput")
    tconst_d = nc.dram_tensor("tconst", [128, NTT * 10], dt.float32,
                              kind="ExternalInput")
    feats_d = nc.dram_tensor("feats", [10, QPC], dt.float32,
                             kind="ExternalInput")
    idxmat_d = nc.dram_tensor("idxmat", [128, NTT], dt.float32,
                              kind="ExternalInput")
    ident_d = nc.dram_tensor("ident", [128, 128], dt.float32,
                             kind="ExternalInput")
    obest_d = nc.dram_tensor("obest", [128, QPC // 128], dt.float32,
                             kind="ExternalOutput")
    oidx_d = nc.dram_tensor("oidx", [128, QPC // 128], dt.float32,
                            kind="ExternalOutput")

    with TileContext(nc) as tc:
        with (
            tc.tile_pool(name="persist", bufs=1) as pp,
            tc.tile_pool(name="coefp", bufs=3) as coefp,
            tc.tile_pool(name="work", bufs=2) as wp,
            tc.tile_pool(name="qp", bufs=2) as qp,
            tc.tile_pool(name="psum", bufs=7, space="PSUM") as psp,
            tc.tile_pool(name="psum1", bufs=1, space="PSUM") as psp1,
        ):
            feats = pp.tile([10, QPC], dt.float32)
            tconst = pp.tile([128, NTT * 10], dt.float32)
            idxmat = pp.tile([128, NTT], dt.float32)
            zeros = pp.tile([128, FD], dt.float32)
            ones = pp.tile([128, FD], dt.float32)
            best = pp.tile([128, QPC], dt.float32)
            bidx = pp.tile([128, QPC], dt.float32)
            ident = pp.tile([128, 128], dt.float32)
            odist = pp.tile([128, QPC // 128], dt.float32)
            oidxs = pp.tile([128, QPC // 128], dt.float32)
            nc.sync.dma_start(ident[:], ident_d[:])

            nc.sync.dma_start(feats[:], feats_d[:])
            nc.sync.dma_start(tconst[:], tconst_d[:])
            nc.sync.dma_start(idxmat[:], idxmat_d[:])
            nc.vector.memset(zeros[:], 0.0)
            nc.vector.memset(ones[:], 1.0)
            nc.vector.memset(best[:], 1.0e30)
            nc.vector.memset(bidx[:], 0.0)

            for t in range(NTT):
                coef_t = coefp.tile([10, NQ * 128], dt.float32, tag="coef")
                nc.sync.dma_start(coef_t[:], coef_d[t])
                # Matmult supports a single sync-wait slot; absorb the coef
                # DMA wait into a throwaway matmul so real matmuls only ever
                # carry their PSUM WAR wait.
                dps = psp1.tile([1, 1], dt.float32, tag="dummy_ps")
                dummy_mm = nc.tensor.matmul(dps[:], coef_t[0:10, 0:1],
                                            coef_t[0:10, 0:1],
                                            start=True, stop=True)
                tc_off = t * 10
                inv_ab2 = tconst[:, tc_off + 0:tc_off + 1]
                inv_ac2 = tconst[:, tc_off + 1:tc_off + 2]
                inv_cb2 = tconst[:, tc_off + 2:tc_off + 3]
                inv_den = tconst[:, tc_off + 3:tc_off + 4]
                abk = [tconst[:, tc_off + 4 + k:tc_off + 5 + k] for k in range(3)]
                ack = [tconst[:, tc_off + 7 + k:tc_off + 8 + k] for k in range(3)]

                for ch in range(NCH):
                    rhs = feats[:, ch * FD:(ch + 1) * FD]
                    q = []
                    ps_keep = {}
                    for qi in range(NQ):
                        ps = psp.tile([128, FD], dt.float32, tag="ps")
                        mm = nc.tensor.matmul(ps[:],
                                              coef_t[:, qi * 128:(qi + 1) * 128],
                                              rhs, start=True, stop=True)
                        if ch == 0 and qi == 0:
                            bass._add_dep_helper(mm.ins, dummy_mm.ins,
                                                 sync=False,
                                                 reason="dummy absorbs coef wait")
                        # d-quantities only feed sign tests -> bf16 SBUF copies
                        # unlock the DVE 16-bit 2x mode for the mask ops (sign
                        # is preserved; +-0 boundary flips are value-continuous)
                        qdt = dt.bfloat16 if qi < 6 else dt.float32
                        sb = qp.tile([128, FD], qdt, tag=f"q{qi}")
                        nc.scalar.copy(sb[:], ps[:])
                        q.append(sb)
                        if qi in (0, 1, 4):
                            ps_keep[qi] = ps
                    (d1, d2, d3, d6, d43, d56, va, vb, vc,
                     apx, apy, apz) = q

                    # scalar parameters on ScalarE (identity with AP scale)
                    t_ab = wp.tile([128, FD], dt.float32, tag="t_ab")
                    w_ac = wp.tile([128, FD], dt.float32, tag="w_ac")
                    t_bc = wp.tile([128, FD], dt.float32, tag="t_bc")
                    one_m = wp.tile([128, FD], dt.float32, tag="one_m")
                    lam = wp.tile([128, FD], dt.float32, tag="lam")
                    mu = wp.tile([128, FD], dt.float32, tag="mu")
                    nc.scalar.activation(t_ab[:], ps_keep[0][:], Act.Copy, scale=inv_ab2)
                    nc.scalar.activation(w_ac[:], ps_keep[1][:], Act.Copy, scale=inv_ac2)
                    nc.scalar.activation(t_bc[:], ps_keep[4][:], Act.Copy, scale=inv_cb2)
                    nc.scalar.activation(lam[:], vb[:], Act.Copy, scale=inv_den)
                    nc.scalar.activation(mu[:], vc[:], Act.Copy, scale=inv_den)
                    nc.scalar.activation(one_m[:], t_bc[:], Act.Copy,
                                         scale=-1.0, bias=1.0)

                    # region masks + predicated lambda/mu updates
                    cmp = wp.tile([128, FD], dt.bfloat16, tag="cmp")
                    tmp = wp.tile([128, FD], dt.bfloat16, tag="tmp")
                    msk = wp.tile([128, FD], dt.bfloat16, tag="msk")

                    def ts(out, in0, s1, op0, s2=None, op1=None):
                        nc.vector.tensor_scalar(out[:], in0[:], s1, s2, op0,
                                                *( [op1] if op1 is not None else []))

                    def stt(out, in0, s, in1, op0, op1):
                        nc.vector.scalar_tensor_tensor(out[:], in0[:], s, in1[:],
                                                       op0, op1)

                    vstt = stt

                    def cpred(out_ap, mask, data_ap):
                        bdt = (dt.uint16 if mask[:].dtype == dt.bfloat16
                               else dt.uint32)
                        nc.vector.copy_predicated(
                            out_ap, mask[:].bitcast(bdt), data_ap)

                    # m_bc = (va<=0)&(d43>=0)&(d56>=0)
                    nc.scalar.activation(cmp[:], va[:], Act.Relu, scale=-1.0)
                    stt(tmp, d43, 0.0, cmp, Alu.is_ge, Alu.logical_and)
                    stt(msk, d56, 0.0, tmp, Alu.is_ge, Alu.logical_and)
                    cpred(lam[:], msk, one_m[:])
                    cpred(mu[:], msk, t_bc[:])
                    # m_ac = (vb<=0)&(d2>=0)&(d6<=0)
                    nc.scalar.activation(cmp[:], vb[:], Act.Relu, scale=-1.0)
                    stt(tmp, d2, 0.0, cmp, Alu.is_ge, Alu.logical_and)
                    stt(msk, d6, 0.0, tmp, Alu.is_le, Alu.logical_and)
                    cpred(lam[:], msk, zeros[:])
                    cpred(mu[:], msk, w_ac[:])
                    # m_ab = (vc<=0)&(d1>=0)&(d3<=0)
                    nc.scalar.activation(cmp[:], vc[:], Act.Relu, scale=-1.0)
                    stt(tmp, d1, 0.0, cmp, Alu.is_ge, Alu.logical_and)
                    stt(msk, d3, 0.0, tmp, Alu.is_le, Alu.logical_and)
                    cpred(lam[:], msk, t_ab[:])
                    cpred(mu[:], msk, zeros[:])
                    # m_c = (d6>=0)&(d56<=0)
                    nc.scalar.activation(cmp[:], d6[:], Act.Relu)
                    stt(msk, d56, 0.0, cmp, Alu.is_le, Alu.logical_and)
                    cpred(lam[:], msk, zeros[:])
                    cpred(mu[:], msk, ones[:])
                    # m_b = (d3>=0)&(d43<=0)
                    nc.scalar.activation(cmp[:], d3[:], Act.Relu)
                    stt(msk, d43, 0.0, cmp, Alu.is_le, Alu.logical_and)
                    cpred(lam[:], msk, ones[:])
                    cpred(mu[:], msk, zeros[:])
                    # m_a = (d1<=0)&(d2<=0)
                    nc.scalar.activation(cmp[:], d1[:], Act.Relu, scale=-1.0)
                    stt(msk, d2, 0.0, cmp, Alu.is_le, Alu.logical_and)
                    cpred(lam[:], msk, zeros[:])
                    cpred(mu[:], msk, zeros[:])

                    # residual squared distance
                    s1 = wp.tile([128, FD], dt.float32, tag="s1")
                    dist = wp.tile([128, FD], dt.float32, tag="dist")
                    sq = []
                    for k, apk in enumerate((apx, apy, apz)):
                        s2t = wp.tile([128, FD], dt.float32, tag=f"s2{k}")
                        stt(s1, lam, abk[k], apk, Alu.mult, Alu.subtract)
                        stt(s2t, mu, ack[k], s1, Alu.mult, Alu.add)
                        sqt = wp.tile([128, FD], dt.float32, tag=f"sq{k}")
                        nc.scalar.activation(sqt[:], s2t[:], Act.Square)
                        sq.append(sqt)
                    nc.vector.tensor_tensor(dist[:], sq[0][:], sq[1][:], Alu.add)
                    nc.vector.tensor_tensor(dist[:], dist[:], sq[2][:], Alu.add)

                    # running min/argmin across triangle tiles
                    upd = wp.tile([128, FD], dt.float32, tag="upd")
                    bsl = best[:, ch * FD:(ch + 1) * FD]
                    isl = bidx[:, ch * FD:(ch + 1) * FD]
                    nc.vector.tensor_tensor(upd[:], dist[:], bsl, Alu.is_lt)
                    cpred(bsl, upd, dist[:])
                    cpred(isl, upd,
                          idxmat[:, t:t + 1].broadcast_to((128, FD)))

            # lane reduction: transpose 128-lane blocks, reduce over lanes
            BIG = 65536.0
            for g in range(QPC // 128):
                tp1 = psp.tile([128, 128], dt.float32, tag="ps")
                nc.tensor.transpose(tp1[:], best[:, g * 128:(g + 1) * 128],
                                    ident[:])
                tp2 = psp.tile([128, 128], dt.float32, tag="ps")
                nc.tensor.transpose(tp2[:], bidx[:, g * 128:(g + 1) * 128],
                                    ident[:])
                nc.vector.tensor_reduce(odist[:, g:g + 1], tp1[:],
                                        axis=mybir.AxisListType.X,
                                        op=Alu.min)
                feq = wp.tile([128, 128], dt.float32, tag="feq")
                nc.vector.tensor_scalar(feq[:], tp1[:], odist[:, g:g + 1],
                                        None, Alu.is_le)
                fc = wp.tile([128, 128], dt.float32, tag="fc")
                nc.vector.scalar_tensor_tensor(fc[:], feq[:], -BIG, tp2[:],
                                               Alu.mult, Alu.add)
                fm = wp.tile([128, 1], dt.float32, tag="fm")
                nc.vector.tensor_reduce(fm[:], fc[:],
                                        axis=mybir.AxisListType.X,
                                        op=Alu.min)
                nc.vector.tensor_scalar(oidxs[:, g:g + 1], fm[:], BIG, None,
                                        Alu.add)
            nc.sync.dma_start(obest_d[:], odist[:])
            nc.sync.dma_start(oidx_d[:], oidxs[:])
    nc.compile()
    return nc


def _get_nc():
    if "nc" not in _CACHE:
        _CACHE["nc"] = _build_bass()
    return _CACHE["nc"]


# ---------------------------------------------------------------- entry point
def kernel(triangles, points):
    from concourse.bass_utils import run_bass_kernel_spmd

    triangles = np.asarray(triangles)
    points = np.asarray(points)

    idxmat = (np.arange(NTT, dtype=F32)[None, :] * 128.0
              + np.arange(128, dtype=F32)[:, None]).astype(F32)
    in_maps = []
    percore = []
    for core in range(NCORES):
        bi = core // CORES_PER_BATCH
        qs = (core % CORES_PER_BATCH) * QPC
        key = f"tri{bi}"
        if key not in _CACHE:
            _CACHE[key] = _tri_precompute(triangles[bi])
        coef, tconst = _CACHE[key]
        pts = points[bi, qs:qs + QPC]
        in_maps.append({
            "coef": coef,
            "tconst": tconst,
            "feats": _point_feats(pts),
            "idxmat": idxmat,
            "ident": np.eye(128, dtype=F32),
        })
        percore.append((bi, qs))

    _CACHE["last_in_maps"] = in_maps
    nc = _get_nc()
    trace = bool(int(os.environ.get("BVH_TRACE", "0")))
    try:
        res = run_bass_kernel_spmd(nc, in_maps, list(range(NCORES)),
                                   trace=trace)
    except ModuleNotFoundError:
        trace = False
        res = run_bass_kernel_spmd(nc, in_maps, list(range(NCORES)),
                                   trace=False)
    if trace:
        _CACHE["exec_time_ns"] = res.exec_time_ns
        _CACHE["profile_json"] = res.profile_json

    distances = np.zeros((B, Q), dtype=F32)
    closest = np.zeros((B, Q, 3), dtype=F32)
    faces = np.zeros((B, Q), dtype=np.int32)
    for core in range(NCORES):
        bi, qs = percore[core]
        oi = res.results[core]["oidx"]           # [128 pts-in-group, 32 groups]
        face = oi.T.reshape(-1).astype(np.int32)  # point P = g*128 + p
        pts = points[bi, qs:qs + QPC].astype(F32)
        tri = triangles[bi, face].astype(F32)    # [QPC, 3, 3]
        cp = _closest_point_host(pts, tri[:, 0], tri[:, 1], tri[:, 2])
        d = np.sum((pts - cp) ** 2, -1).astype(F32)
        distances[bi, qs:qs + QPC] = d
        closest[bi, qs:qs + QPC] = cp
        faces[bi, qs:qs + QPC] = face
    return distances, closest, faces


# revision 15
# speedup vs baseline: 1.5503x; 1.5503x over previous
"""Trainium2 Bass kernel: brute-force closest-point-on-triangle kNN.

Problem: triangles [2,4096,3,3] f32, points [2,16384,3] f32 ->
  (squared distances [2,16384] f32, closest_points [2,16384,3] f32,
   closest_faces [2,16384] int32)

Strategy (data-parallel over points, 8 cores, 4 per batch):
  - Host precomputes, per triangle, polynomial coefficients so that one
    TensorE matmul (K=10 quadratic point-features) produces the 12 per-pair
    quantities of Ericson's closest-point-on-triangle region test:
       d1,d2,d3,d6, d4-d3, d5-d6, va,vb,vc, ap_x,ap_y,ap_z
  - Device (per 128-triangle tile x 512-point chunk): region masks via
    fused compare/and ops, barycentric (lambda,mu) selection via
    copy_predicated chains, squared distance from the residual
    r = ap - lambda*ab - mu*ac, and a running min/argmin per triangle-lane.
  - Host reduces the 128 triangle lanes per point, gathers the winning
    triangle and recomputes the exact closest point/distance with the
    reference formula (fp32), guaranteeing output fidelity.
"""

import os
import numpy as np

F32 = np.float32

B, F, Q = 2, 4096, 16384
NCORES = 8
CORES_PER_BATCH = 4
QPC = Q // CORES_PER_BATCH          # 4096 points per core
NTT = F // 128                      # 32 triangle tiles
FD = 512                            # point chunk (free dim)
NCH = QPC // FD                     # 8 chunks per core
NQ = 12                             # matmul quantities per pair

_CACHE = {}


# ---------------------------------------------------------------- host math
def _tri_precompute(tris_b):
    """Per-triangle coefficients and constants for one batch.

    Returns coef [NTT, 10, NQ*128] f32  (feature k -> partition, quantity q,
    tri-in-tile m -> free q*128+m) and tconst [128, NTT*10] f32
    (partition = tri-in-tile, free = t*10 + const_id) with const order:
    inv_ab2, inv_ac2, inv_cb2, inv_den, ab_xyz, ac_xyz.
    """
    a = tris_b[:, 0].astype(F32)
    b = tris_b[:, 1].astype(F32)
    c = tris_b[:, 2].astype(F32)
    ab = b - a
    ac = c - a
    cb = c - b

    def guarded_inv(x):
        x = x.astype(F32)
        return (1.0 / np.where(np.abs(x) > 1e-12, x, F32(1.0))).astype(F32)

    ab2 = np.sum(ab * ab, -1)
    ac2 = np.sum(ac * ac, -1)
    cb2 = np.sum(cb * cb, -1)
    cr = np.cross(ab.astype(np.float64), ac.astype(np.float64))
    den = np.sum(cr * cr, -1).astype(F32)
    inv_ab2, inv_ac2, inv_cb2, inv_den = (
        guarded_inv(ab2), guarded_inv(ac2), guarded_inv(cb2), guarded_inv(den))

    # coefficient rows over features [1,x,y,z,x2,y2,z2,xy,xz,yz]
    ab64 = ab.astype(np.float64); ac64 = ac.astype(np.float64)
    a64 = a.astype(np.float64); b64 = b.astype(np.float64)
    c64 = c.astype(np.float64); cb64 = cb.astype(np.float64)
    C = np.zeros((F, NQ, 10), dtype=np.float64)

    def aff(q, vec, const):
        C[:, q, 1:4] = vec
        C[:, q, 0] = const

    e1 = -np.sum(ab64 * a64, -1)
    e2 = -np.sum(ac64 * a64, -1)
    e3 = -np.sum(ab64 * b64, -1)
    e6 = -np.sum(ac64 * c64, -1)
    e4 = -np.sum(ac64 * b64, -1)
    e5 = -np.sum(ab64 * c64, -1)
    aff(0, ab64, e1)                      # d1
    aff(1, ac64, e2)                      # d2
    aff(2, ab64, e3)                      # d3
    aff(3, ac64, e6)                      # d6
    aff(4, cb64, -np.sum(cb64 * b64, -1))  # d4-d3
    aff(5, -cb64, np.sum(cb64 * c64, -1))  # d5-d6

    def quad(u, e, v, f):
        out = np.zeros((F, 10), dtype=np.float64)
        out[:, 0] = e * f
        out[:, 1:4] = u * f[:, None] + v * e[:, None]
        out[:, 4:7] = u * v
        out[:, 7] = u[:, 0] * v[:, 1] + u[:, 1] * v[:, 0]
        out[:, 8] = u[:, 0] * v[:, 2] + u[:, 2] * v[:, 0]
        out[:, 9] = u[:, 1] * v[:, 2] + u[:, 2] * v[:, 1]
        return out

    C[:, 6] = quad(ab64, e3, ac64, e6) - quad(ab64, e5, ac64, e4)   # va
    C[:, 7] = quad(ab64, e5, ac64, e2) - quad(ab64, e1, ac64, e6)   # vb
    C[:, 8] = quad(ab64, e1, ac64, e4) - quad(ab64, e3, ac64, e2)   # vc
    for k in range(3):                    # ap_k = p_k - a_k
        C[:, 9 + k, 0] = -a[:, k]
        C[:, 9 + k, 1 + k] = 1.0

    Cf = C.astype(F32)                                   # [F, NQ, 10]
    coef = np.zeros((NTT, 10, NQ * 128), dtype=F32)
    for t in range(NTT):
        blk = Cf[t * 128:(t + 1) * 128]                  # [128, NQ, 10]
        coef[t] = blk.transpose(2, 1, 0).reshape(10, NQ * 128)

    tconst = np.zeros((128, NTT * 10), dtype=F32)
    cs = np.stack([inv_ab2, inv_ac2, inv_cb2, inv_den,
                   ab[:, 0], ab[:, 1], ab[:, 2],
                   ac[:, 0], ac[:, 1], ac[:, 2]], axis=1)  # [F, 10]
    for t in range(NTT):
        tconst[:, t * 10:(t + 1) * 10] = cs[t * 128:(t + 1) * 128]
    return coef, tconst


def _point_feats(pts):
    p = pts.astype(F32)
    x, y, z = p[:, 0], p[:, 1], p[:, 2]
    return np.stack([np.ones_like(x), x, y, z,
                     x * x, y * y, z * z,
                     x * y, x * z, y * z], axis=0).astype(F32)  # [10, n]


def _safe_div(num, den):
    den_s = np.where(np.abs(den) > 1e-12, den, F32(1.0)).astype(F32)
    return (num / den_s).astype(F32)


def _closest_point_host(p, a, b, c):
    """Exact reference formula (fp32) for n points against n triangles."""
    ab = b - a
    ac = c - a
    ap = p - a
    d1 = np.sum(ab * ap, -1)
    d2 = np.sum(ac * ap, -1)
    bp = p - b
    d3 = np.sum(ab * bp, -1)
    d4 = np.sum(ac * bp, -1)
    cpv = p - c
    d5 = np.sum(ab * cpv, -1)
    d6 = np.sum(ac * cpv, -1)
    vc = d1 * d4 - d3 * d2
    vb = d5 * d2 - d1 * d6
    va = d3 * d6 - d5 * d4
    v_ab = np.clip(_safe_div(d1, d1 - d3), 0.0, 1.0).astype(F32)
    w_ac = np.clip(_safe_div(d2, d2 - d6), 0.0, 1.0).astype(F32)
    w_bc = np.clip(_safe_div(d4 - d3, (d4 - d3) + (d5 - d6)), 0.0, 1.0).astype(F32)
    denom = va + vb + vc
    v = _safe_div(vb, denom)
    w = _safe_div(vc, denom)
    res = a + ab * v[..., None] + ac * w[..., None]
    m_bc = (va <= 0) & ((d4 - d3) >= 0) & ((d5 - d6) >= 0)
    m_ac = (vb <= 0) & (d2 >= 0) & (d6 <= 0)
    m_ab = (vc <= 0) & (d1 >= 0) & (d3 <= 0)
    m_c = (d6 >= 0) & (d5 <= d6)
    m_b = (d3 >= 0) & (d4 <= d3)
    m_a = (d1 <= 0) & (d2 <= 0)
    res = np.where(m_bc[..., None], b + w_bc[..., None] * (c - b), res)
    res = np.where(m_ac[..., None], a + w_ac[..., None] * ac, res)
    res = np.where(m_ab[..., None], a + v_ab[..., None] * ab, res)
    res = np.where(m_c[..., None], c, res)
    res = np.where(m_b[..., None], b, res)
    res = np.where(m_a[..., None], a, res)
    return res.astype(F32)


# ---------------------------------------------------------------- bass build
def _build_bass():
    import concourse.bass as bass
    import concourse.bacc as bacc
    import concourse.mybir as mybir
    from concourse.tile import TileContext

    dt = mybir.dt
    Alu = mybir.AluOpType
    Act = mybir.ActivationFunctionType

    nc = bacc.Bacc()
    coef_d = nc.dram_tensor("coef", [NTT, 10, NQ * 128], dt.float32,
                            kind="ExternalInput")
    tconst_d = nc.dram_tensor("tconst", [128, NTT * 10], dt.float32,
                              kind="ExternalInput")
    feats_d = nc.dram_tensor("feats", [10, QPC], dt.float32,
                             kind="ExternalInput")
    idxmat_d = nc.dram_tensor("idxmat", [128, NTT], dt.float32,
                              kind="ExternalInput")
    ident_d = nc.dram_tensor("ident", [128, 128], dt.float32,
                             kind="ExternalInput")
    obest_d = nc.dram_tensor("obest", [128, QPC // 128], dt.float32,
                             kind="ExternalOutput")
    oidx_d = nc.dram_tensor("oidx", [128, QPC // 128], dt.float32,
                            kind="ExternalOutput")

    with TileContext(nc) as tc:
        with (
            tc.tile_pool(name="persist", bufs=1) as pp,
            tc.tile_pool(name="coefp", bufs=3) as coefp,
            tc.tile_pool(name="work", bufs=2) as wp,
            tc.tile_pool(name="qp", bufs=2) as qp,
            tc.tile_pool(name="psum", bufs=7, space="PSUM") as psp,
            tc.tile_pool(name="psum1", bufs=1, space="PSUM") as psp1,
        ):
            feats = pp.tile([10, QPC], dt.float32)
            tconst = pp.tile([128, NTT * 10], dt.float32)
            idxmat = pp.tile([128, NTT], dt.float32)
            zeros = pp.tile([128, FD], dt.float32)
            ones = pp.tile([128, FD], dt.float32)
            best = pp.tile([128, QPC], dt.float32)
            bidx = pp.tile([128, QPC], dt.float32)
            ident = pp.tile([128, 128], dt.float32)
            odist = pp.tile([128, QPC // 128], dt.float32)
            oidxs = pp.tile([128, QPC // 128], dt.float32)
            nc.sync.dma_start(ident[:], ident_d[:])

            nc.sync.dma_start(feats[:], feats_d[:])
            nc.sync.dma_start(tconst[:], tconst_d[:])
            nc.sync.dma_start(idxmat[:], idxmat_d[:])
            nc.vector.memset(zeros[:], 0.0)
            nc.vector.memset(ones[:], 1.0)
            nc.vector.memset(best[:], 1.0e30)
            nc.vector.memset(bidx[:], 0.0)

            for t in range(NTT):
                coef_t = coefp.tile([10, NQ * 128], dt.float32, tag="coef")
                nc.sync.dma_start(coef_t[:], coef_d[t])
                # Matmult supports a single sync-wait slot; absorb the coef
                # DMA wait into a throwaway matmul so real matmuls only ever
                # carry their PSUM WAR wait.
                dps = psp1.tile([1, 1], dt.float32, tag="dummy_ps")
                dummy_mm = nc.tensor.matmul(dps[:], coef_t[0:10, 0:1],
                                            coef_t[0:10, 0:1],
                                            start=True, stop=True)
                tc_off = t * 10
                inv_ab2 = tconst[:, tc_off + 0:tc_off + 1]
                inv_ac2 = tconst[:, tc_off + 1:tc_off + 2]
                inv_cb2 = tconst[:, tc_off + 2:tc_off + 3]
                inv_den = tconst[:, tc_off + 3:tc_off + 4]
                abk = [tconst[:, tc_off + 4 + k:tc_off + 5 + k] for k in range(3)]
                ack = [tconst[:, tc_off + 7 + k:tc_off + 8 + k] for k in range(3)]

                for ch in range(NCH):
                    rhs = feats[:, ch * FD:(ch + 1) * FD]
                    q = []
                    ps_keep = {}
                    for qi in range(NQ):
                        ps = psp.tile([128, FD], dt.float32, tag="ps")
                        mm = nc.tensor.matmul(ps[:],
                                              coef_t[:, qi * 128:(qi + 1) * 128],
                                              rhs, start=True, stop=True)
                        if ch == 0 and qi == 0:
                            bass._add_dep_helper(mm.ins, dummy_mm.ins,
                                                 sync=False,
                                                 reason="dummy absorbs coef wait")
                        # d-quantities only feed sign tests -> bf16 SBUF copies
                        # unlock the DVE 16-bit 2x mode for the mask ops (sign
                        # is preserved; +-0 boundary flips are value-continuous)
                        qdt = dt.bfloat16 if qi < 6 else dt.float32
                        sb = qp.tile([128, FD], qdt, tag=f"q{qi}")
                        nc.scalar.copy(sb[:], ps[:])
                        q.append(sb)
                        if qi in (0, 1, 4):
                            ps_keep[qi] = ps
                    (d1, d2, d3, d6, d43, d56, va, vb, vc,
                     apx, apy, apz) = q

                    # scalar parameters on ScalarE (identity with AP scale)
                    t_ab = wp.tile([128, FD], dt.float32, tag="t_ab")
                    w_ac = wp.tile([128, FD], dt.float32, tag="w_ac")
                    t_bc = wp.tile([128, FD], dt.float32, tag="t_bc")
                    one_m = wp.tile([128, FD], dt.float32, tag="one_m")
                    lam = wp.tile([128, FD], dt.float32, tag="lam")
                    mu = wp.tile([128, FD], dt.float32, tag="mu")
                    nc.scalar.activation(t_ab[:], ps_keep[0][:], Act.Copy, scale=inv_ab2)
                    nc.scalar.activation(w_ac[:], ps_keep[1][:], Act.Copy, scale=inv_ac2)
                    nc.scalar.activation(t_bc[:], ps_keep[4][:], Act.Copy, scale=inv_cb2)
                    nc.scalar.activation(lam[:], vb[:], Act.Copy, scale=inv_den)
                    nc.scalar.activation(mu[:], vc[:], Act.Copy, scale=inv_den)
                    nc.scalar.activation(one_m[:], t_bc[:], Act.Copy,
                                         scale=-1.0, bias=1.0)

                    # region masks + predicated lambda/mu updates
                    cmp = wp.tile([128, FD], dt.bfloat16, tag="cmp")
                    tmp = wp.tile([128, FD], dt.bfloat16, tag="tmp")
                    msk = wp.tile([128, FD], dt.bfloat16, tag="msk")

                    def ts(out, in0, s1, op0, s2=None, op1=None):
                        nc.vector.tensor_scalar(out[:], in0[:], s1, s2, op0,
                                                *( [op1] if op1 is not None else []))

                    def stt(out, in0, s, in1, op0, op1):
                        nc.vector.scalar_tensor_tensor(out[:], in0[:], s, in1[:],
                                                       op0, op1)

                    vstt = stt

                    def cpred(out_ap, mask, data_ap):
                        bdt = (dt.uint16 if mask[:].dtype == dt.bfloat16
                               else dt.uint32)
                        nc.vector.copy_predicated(
                            out_ap, mask[:].bitcast(bdt), data_ap)

                    # Region masks kept live; zero-assignments of the
                    # lambda/mu select chains are grouped into one
                    # copy_predicated via OR-combined masks (reorder differs
                    # only on degenerate region overlaps, where the closest
                    # point is continuous; 0 argmin flips on real data).
                    mbc = wp.tile([128, FD], dt.bfloat16, tag="mbc")
                    mac = wp.tile([128, FD], dt.bfloat16, tag="mac")
                    mab = wp.tile([128, FD], dt.bfloat16, tag="mab")
                    mc = wp.tile([128, FD], dt.bfloat16, tag="mc")
                    mb = wp.tile([128, FD], dt.bfloat16, tag="mb")
                    ma = wp.tile([128, FD], dt.bfloat16, tag="ma")
                    zl = wp.tile([128, FD], dt.bfloat16, tag="zl")
                    zm = wp.tile([128, FD], dt.bfloat16, tag="zm")
                    # m_bc = (va<=0)&(d43>=0)&(d56>=0)
                    nc.scalar.activation(cmp[:], va[:], Act.Relu, scale=-1.0)
                    stt(tmp, d43, 0.0, cmp, Alu.is_ge, Alu.logical_and)
                    stt(mbc, d56, 0.0, tmp, Alu.is_ge, Alu.logical_and)
                    # m_ac = (vb<=0)&(d2>=0)&(d6<=0)
                    nc.scalar.activation(cmp[:], vb[:], Act.Relu, scale=-1.0)
                    stt(tmp, d2, 0.0, cmp, Alu.is_ge, Alu.logical_and)
                    stt(mac, d6, 0.0, tmp, Alu.is_le, Alu.logical_and)
                    # m_ab = (vc<=0)&(d1>=0)&(d3<=0)
                    nc.scalar.activation(cmp[:], vc[:], Act.Relu, scale=-1.0)
                    stt(tmp, d1, 0.0, cmp, Alu.is_ge, Alu.logical_and)
                    stt(mab, d3, 0.0, tmp, Alu.is_le, Alu.logical_and)
                    # m_c = (d6>=0)&(d56<=0)
                    nc.scalar.activation(cmp[:], d6[:], Act.Relu)
                    stt(mc, d56, 0.0, cmp, Alu.is_le, Alu.logical_and)
                    # m_b = (d3>=0)&(d43<=0)
                    nc.scalar.activation(cmp[:], d3[:], Act.Relu)
                    stt(mb, d43, 0.0, cmp, Alu.is_le, Alu.logical_and)
                    # m_a = (d1<=0)&(d2<=0)
                    nc.scalar.activation(cmp[:], d1[:], Act.Relu, scale=-1.0)
                    stt(ma, d2, 0.0, cmp, Alu.is_le, Alu.logical_and)
                    # lambda: nonzero updates in priority order, zeros last
                    cpred(lam[:], mbc, one_m[:])
                    cpred(lam[:], mab, t_ab[:])
                    cpred(lam[:], mb, ones[:])
                    nc.vector.tensor_tensor(zl[:], mac[:], mc[:], Alu.logical_or)
                    nc.vector.tensor_tensor(zl[:], zl[:], ma[:], Alu.logical_or)
                    cpred(lam[:], zl, zeros[:])
                    # mu
                    cpred(mu[:], mbc, t_bc[:])
                    cpred(mu[:], mac, w_ac[:])
                    cpred(mu[:], mc, ones[:])
                    nc.vector.tensor_tensor(zm[:], mab[:], mb[:], Alu.logical_or)
                    nc.vector.tensor_tensor(zm[:], zm[:], ma[:], Alu.logical_or)
                    cpred(mu[:], zm, zeros[:])

                    # residual squared distance
                    s1 = wp.tile([128, FD], dt.float32, tag="s1")
                    dist = wp.tile([128, FD], dt.float32, tag="dist")
                    sq = []
                    for k, apk in enumerate((apx, apy, apz)):
                        s2t = wp.tile([128, FD], dt.float32, tag=f"s2{k}")
                        stt(s1, lam, abk[k], apk, Alu.mult, Alu.subtract)
                        stt(s2t, mu, ack[k], s1, Alu.mult, Alu.add)
                        sqt = wp.tile([128, FD], dt.float32, tag=f"sq{k}")
                        nc.scalar.activation(sqt[:], s2t[:], Act.Square)
                        sq.append(sqt)
                    nc.vector.tensor_tensor(dist[:], sq[0][:], sq[1][:], Alu.add)
                    nc.vector.tensor_tensor(dist[:], dist[:], sq[2][:], Alu.add)

                    # running min/argmin across triangle tiles
                    upd = wp.tile([128, FD], dt.float32, tag="upd")
                    bsl = best[:, ch * FD:(ch + 1) * FD]
                    isl = bidx[:, ch * FD:(ch + 1) * FD]
                    nc.vector.tensor_tensor(upd[:], dist[:], bsl, Alu.is_lt)
                    cpred(bsl, upd, dist[:])
                    cpred(isl, upd,
                          idxmat[:, t:t + 1].broadcast_to((128, FD)))

            # lane reduction: transpose 128-lane blocks, reduce over lanes
            BIG = 65536.0
            for g in range(QPC // 128):
                tp1 = psp.tile([128, 128], dt.float32, tag="ps")
                nc.tensor.transpose(tp1[:], best[:, g * 128:(g + 1) * 128],
                                    ident[:])
                tp2 = psp.tile([128, 128], dt.float32, tag="ps")
                nc.tensor.transpose(tp2[:], bidx[:, g * 128:(g + 1) * 128],
                                    ident[:])
                nc.vector.tensor_reduce(odist[:, g:g + 1], tp1[:],
                                        axis=mybir.AxisListType.X,
                                        op=Alu.min)
                feq = wp.tile([128, 128], dt.float32, tag="feq")
                nc.vector.tensor_scalar(feq[:], tp1[:], odist[:, g:g + 1],
                                        None, Alu.is_le)
                fc = wp.tile([128, 128], dt.float32, tag="fc")
                nc.vector.scalar_tensor_tensor(fc[:], feq[:], -BIG, tp2[:],
                                               Alu.mult, Alu.add)
                fm = wp.tile([128, 1], dt.float32, tag="fm")
                nc.vector.tensor_reduce(fm[:], fc[:],
                                        axis=mybir.AxisListType.X,
                                        op=Alu.min)
                nc.vector.tensor_scalar(oidxs[:, g:g + 1], fm[:], BIG, None,
                                        Alu.add)
            nc.sync.dma_start(obest_d[:], odist[:])
            nc.sync.dma_start(oidx_d[:], oidxs[:])
    nc.compile()
    return nc


def _get_nc():
    if "nc" not in _CACHE:
        _CACHE["nc"] = _build_bass()
    return _CACHE["nc"]


# ---------------------------------------------------------------- entry point
def kernel(triangles, points):
    from concourse.bass_utils import run_bass_kernel_spmd

    triangles = np.asarray(triangles)
    points = np.asarray(points)

    idxmat = (np.arange(NTT, dtype=F32)[None, :] * 128.0
              + np.arange(128, dtype=F32)[:, None]).astype(F32)
    in_maps = []
    percore = []
    for core in range(NCORES):
        bi = core // CORES_PER_BATCH
        qs = (core % CORES_PER_BATCH) * QPC
        key = f"tri{bi}"
        if key not in _CACHE:
            _CACHE[key] = _tri_precompute(triangles[bi])
        coef, tconst = _CACHE[key]
        pts = points[bi, qs:qs + QPC]
        in_maps.append({
            "coef": coef,
            "tconst": tconst,
            "feats": _point_feats(pts),
            "idxmat": idxmat,
            "ident": np.eye(128, dtype=F32),
        })
        percore.append((bi, qs))

    _CACHE["last_in_maps"] = in_maps
    nc = _get_nc()
    trace = bool(int(os.environ.get("BVH_TRACE", "0")))
    try:
        res = run_bass_kernel_spmd(nc, in_maps, list(range(NCORES)),
                                   trace=trace)
    except ModuleNotFoundError:
        trace = False
        res = run_bass_kernel_spmd(nc, in_maps, list(range(NCORES)),
                                   trace=False)
    if trace:
        _CACHE["exec_time_ns"] = res.exec_time_ns
        _CACHE["profile_json"] = res.profile_json

    distances = np.zeros((B, Q), dtype=F32)
    closest = np.zeros((B, Q, 3), dtype=F32)
    faces = np.zeros((B, Q), dtype=np.int32)
    for core in range(NCORES):
        bi, qs = percore[core]
        oi = res.results[core]["oidx"]           # [128 pts-in-group, 32 groups]
        face = oi.T.reshape(-1).astype(np.int32)  # point P = g*128 + p
        pts = points[bi, qs:qs + QPC].astype(F32)
        tri = triangles[bi, face].astype(F32)    # [QPC, 3, 3]
        cp = _closest_point_host(pts, tri[:, 0], tri[:, 1], tri[:, 2])
        d = np.sum((pts - cp) ** 2, -1).astype(F32)
        distances[bi, qs:qs + QPC] = d
        closest[bi, qs:qs + QPC] = cp
        faces[bi, qs:qs + QPC] = face
    return distances, closest, faces
